# revision 1
# baseline (speedup 1.0000x reference)
"""GravityField Trainium2 kernel.

out[b,t,i,j] = G[b,t,i,j] + 0.1*grav[b,t]*(i==j)
  grav = (phi @ phi_sum), phi = sqrt(2/R) cos(coords@W + b),
  phi_sum = sum_t phi*mass, mass = softplus(relu(coords@w1.T+b1)@w2.T+b2)

Strategy: data-parallel over B (8 cores, 1 batch each). Per core:
  - tiny prologue on TensorE/ACT/DVE computes grav[t] for its 4096 tokens
    (cos via range-reduced Sin; softplus via Ln(1+Exp));
  - main loop streams G (64 MB) through SBUF in [128, 4096] tiles
    (partition p holds one 64x64 matrix) and adds grav[t] to the 64
    diagonal elements via one strided tensor_scalar, then streams out.
    Pure HBM-bandwidth bound; 8 tile buffers prefetch G under the
    prologue so the store pipeline starts as soon as grav is ready.
"""

import sys

for p in ("/opt/trn_rl_repo", "/opt/pypackages"):
    if p not in sys.path:
        sys.path.insert(0, p)

import numpy as np

B, T, D, R = 8, 4096, 64, 64
STRENGTH = 0.1
N_CORES = 8
TOK_TILE = 128            # tokens per G tile (one per partition)
N_TILES = T // TOK_TILE   # 32 G tiles per core
GBUFS = 8
CHUNK = 512               # prologue token chunk (1 PSUM bank)
N_CHUNKS = T // CHUNK
GRAV_COPY_GROUP = 4       # psum->sbuf gravc copy granularity (tiles)
MAGIC = np.float32(1.5 * 2**23)   # fp32 round-to-nearest-integer trick
TWO_PI = float(2.0 * np.pi)
INV_2PI = float(1.0 / (2.0 * np.pi))
# grav addend scale: STRENGTH * (sqrt(2/R))^2 folded into one constant
GSCALE = float(STRENGTH * 2.0 / R)

_CACHE = {}


def _build(repeat=1):
    import concourse.bacc as bacc
    import concourse.mybir as mybir
    import concourse.tile as tile

    f32 = mybir.dt.float32
    AF = mybir.ActivationFunctionType

    # Pin the activation-table chooser to two sets: Relu/Exp/Ln/Identity
    # all live in natural_log_exp_and_others and Sin in trig_and_small.
    # Without this the greedy chooser alternates between sets that hold
    # only one of Exp/Ln each (15 table loads ~ 19 us on the ACT engine).
    # Set names and order are preserved, so act_func_set_id stays a valid
    # index into act_info.json.
    KEEP = {"natural_log_exp_and_others", "trig_and_small"}
    MINE = {AF.Relu, AF.Exp, AF.Ln, AF.Sin, AF.Identity, AF.Copy}
    orig_tables = bacc.get_activation_tables

    def pruned_tables(arch):
        t = orig_tables(arch)
        return {name: (fns if name in KEEP else (fns - MINE))
                for name, fns in t.items()}

    nc = bacc.Bacc("TRN2", target_bir_lowering=False, debug=False,
                   enable_asserts=False, num_devices=N_CORES)

    g_in = nc.dram_tensor("g", [T, D * D], f32, kind="ExternalInput")
    ct_in = nc.dram_tensor("ct", [D, T], f32, kind="ExternalInput")
    w1t_in = nc.dram_tensor("w1t", [D, D], f32, kind="ExternalInput")
    w2r_in = nc.dram_tensor("w2r", [D, D], f32, kind="ExternalInput")
    wrf_in = nc.dram_tensor("wrf", [D, R], f32, kind="ExternalInput")
    b1_in = nc.dram_tensor("b1c", [D, 1], f32, kind="ExternalInput")
    bph_in = nc.dram_tensor("bph", [R, 1], f32, kind="ExternalInput")
    b2_in = nc.dram_tensor("b2s", [D, 1], f32, kind="ExternalInput")
    out = nc.dram_tensor("out", [T, D * D], f32, kind="ExternalOutput")

    with tile.TileContext(nc) as tc:
        with (
            tc.tile_pool(name="const", bufs=1) as cpool,
            tc.tile_pool(name="work", bufs=2) as wpool,
            tc.tile_pool(name="psum", bufs=2, space="PSUM") as ppool,
            tc.tile_pool(name="gpsum", bufs=1, space="PSUM") as gppool,
            tc.tile_pool(name="gtiles", bufs=GBUFS) as gpool,
        ):
          for _rep in range(repeat):
            # ---- persistent small tensors ----
            ct = cpool.tile([D, T], f32)
            nc.sync.dma_start(out=ct[:], in_=ct_in[:])
            w1t = cpool.tile([D, D], f32)
            nc.sync.dma_start(out=w1t[:], in_=w1t_in[:])
            w2r = cpool.tile([D, D], f32)
            nc.sync.dma_start(out=w2r[:], in_=w2r_in[:])
            wrf = cpool.tile([D, R], f32)
            nc.sync.dma_start(out=wrf[:], in_=wrf_in[:])
            b1c = cpool.tile([D, 1], f32)
            nc.sync.dma_start(out=b1c[:], in_=b1_in[:])
            bph = cpool.tile([R, 1], f32)
            nc.sync.dma_start(out=bph[:], in_=bph_in[:])
            b2s = cpool.tile([D, 1], f32)
            nc.sync.dma_start(out=b2s[:], in_=b2_in[:])
            phiT = cpool.tile([R, T], f32)
            partials = cpool.tile([R, N_CHUNKS], f32)
            phisum = cpool.tile([R, 1], f32)
            gravc = cpool.tile([128, N_TILES], f32)

            # ---- phase B: phi (ACT: Sin only -> trig table) ----
            for c in range(N_CHUNKS):
                sl = slice(c * CHUNK, (c + 1) * CHUNK)
                pz = ppool.tile([R, CHUNK], f32, tag="pz")
                nc.tensor.matmul(pz[:], wrf[:], ct[:, sl])
                u = wpool.tile([R, CHUNK], f32, tag="u")
                # u = z/(2pi) + (b + pi/2)/(2pi), one DVE op from PSUM
                nc.vector.tensor_scalar(out=u[:], in0=pz[:],
                                        scalar1=INV_2PI, scalar2=bph[:],
                                        op0=mybir.AluOpType.mult,
                                        op1=mybir.AluOpType.add)
                n = wpool.tile([R, CHUNK], f32, tag="n")
                nc.vector.tensor_scalar_add(out=n[:], in0=u[:],
                                            scalar1=float(MAGIC))
                nc.vector.tensor_scalar_add(out=n[:], in0=n[:],
                                            scalar1=-float(MAGIC))
                r_ = wpool.tile([R, CHUNK], f32, tag="r_")
                nc.vector.tensor_tensor(out=r_[:], in0=u[:], in1=n[:],
                                        op=mybir.AluOpType.subtract)
                nc.scalar.activation(out=phiT[:, sl], in_=r_[:], func=AF.Sin,
                                     scale=TWO_PI)

            # ---- phase A: mass (ACT: Relu/Exp/Ln -> one table) + partials
            for c in range(N_CHUNKS):
                sl = slice(c * CHUNK, (c + 1) * CHUNK)
                ph = ppool.tile([D, CHUNK], f32, tag="ph")
                nc.tensor.matmul(ph[:], w1t[:], ct[:, sl])
                h = wpool.tile([D, CHUNK], f32, tag="h")
                nc.scalar.activation(out=h[:], in_=ph[:], func=AF.Relu,
                                     bias=b1c[:])
                pm = ppool.tile([D, CHUNK], f32, tag="pm")
                nc.tensor.matmul(pm[:], w2r[:], h[:])
                me = wpool.tile([D, CHUNK], f32, tag="me")
                nc.scalar.activation(out=me[:], in_=pm[:], func=AF.Exp,
                                     bias=b2s[:])
                ms = wpool.tile([D, CHUNK], f32, tag="ms")
                nc.scalar.activation(out=ms[:], in_=me[:], func=AF.Ln,
                                     bias=1.0)
                pmu = wpool.tile([R, CHUNK], f32, tag="pmu")
                nc.vector.tensor_tensor(out=pmu[:], in0=phiT[:, sl],
                                        in1=ms[:], op=mybir.AluOpType.mult)
                nc.vector.tensor_reduce(out=partials[:, c:c + 1], in_=pmu[:],
                                        axis=mybir.AxisListType.X,
                                        op=mybir.AluOpType.add)

            # ---- phi_sum and per-token grav ----
            nc.vector.tensor_reduce(out=phisum[:], in_=partials[:],
                                    axis=mybir.AxisListType.X,
                                    op=mybir.AluOpType.add)
            pg = gppool.tile([128, N_TILES], f32)
            for k in range(N_TILES):
                lhs = phiT[:, k * TOK_TILE:(k + 1) * TOK_TILE]
                nc.tensor.matmul(pg[:, k:k + 1], lhs, phisum[:])
                if (k + 1) % GRAV_COPY_GROUP == 0:
                    lo = k + 1 - GRAV_COPY_GROUP
                    nc.scalar.activation(out=gravc[:, lo:k + 1],
                                         in_=pg[:, lo:k + 1], func=AF.Copy,
                                         scale=GSCALE)

            # ---- main loop: stream G, add grav to diagonals ----
            for k in range(N_TILES):
                rows = g_in[k * TOK_TILE:(k + 1) * TOK_TILE, :]
                orows = out[k * TOK_TILE:(k + 1) * TOK_TILE, :]
                gt = gpool.tile([128, D * D], f32, tag="gt")
                nc.sync.dma_start(out=gt[:], in_=rows)
                diag = gt[:, 0:D * D:D + 1]
                nc.vector.tensor_scalar_add(out=diag, in0=diag,
                                            scalar1=gravc[:, k:k + 1])
                nc.sync.dma_start(out=orows, in_=gt[:])

    bacc.get_activation_tables = pruned_tables
    try:
        nc.compile()
    finally:
        bacc.get_activation_tables = orig_tables
    return nc


def kernel(G, coords, w1, b1, w2, b2, W, b, **extra):
    from concourse.bass_utils import run_bass_kernel_spmd

    if "nc" not in _CACHE:
        _CACHE["nc"] = _build()
    nc = _CACHE["nc"]

    w1t = np.ascontiguousarray(w1.astype(np.float32).T)
    w2r = np.ascontiguousarray(np.tile(np.asarray(w2, np.float32).reshape(D, 1), (1, D)))
    wrf = np.ascontiguousarray(np.asarray(W, np.float32))
    b1c = np.ascontiguousarray(np.asarray(b1, np.float32).reshape(D, 1))
    bph = np.ascontiguousarray(
        ((np.asarray(b, np.float64) + np.pi / 2) / (2 * np.pi))
        .astype(np.float32).reshape(R, 1))
    b2s = np.full((D, 1), float(np.asarray(b2).reshape(-1)[0]), np.float32)

    in_maps = []
    for core in range(N_CORES):
        in_maps.append({
            "g": np.ascontiguousarray(G[core], np.float32).reshape(T, D * D),
            "ct": np.ascontiguousarray(np.asarray(coords[core], np.float32).T),
            "w1t": w1t, "w2r": w2r, "wrf": wrf,
            "b1c": b1c, "bph": bph, "b2s": b2s,
        })

    res = run_bass_kernel_spmd(nc, in_maps, list(range(N_CORES)))
    out = np.empty((B, T, D, D), dtype=np.float32)
    for core in range(N_CORES):
        out[core] = res.results[core]["out"].reshape(T, D, D)
    return out

